# revision 1
# baseline (speedup 1.0000x reference)
"""Trainium2 Bass kernel for nn_KeyedConv2d: 3x3 SAME conv, stride 1.

x: [8, 64, 64, 64] (NCHW), Wt: [64, 64, 3, 3] (OIHW) -> out [8, 64, 64, 64].

Sharding: data-parallel over batch, one image per NeuronCore (8 cores).

Per-core algorithm: conv = sum over the 9 kernel offsets of a [IC=64 x OC=64]
matmul applied to a shifted view of the zero-padded image held in SBUF.
The padded image [64, 66*66] is duplicated into both SBUF partition halves so
two output chunks (512 pixels each) run concurrently on the two 64-row strips
of the PE array (tile_position row packing; fp32r forbids column packing).
Each strip accumulates its chunk's 9 offsets into its own PSUM bank; DVE
copies PSUM->SBUF and DMA stores to HBM.  Matmuls run in float32r (full PE
rate; ~1e-4 scaled error) -- set MODE="f32" for exact-but-4x-slower matmuls.
"""
import numpy as np

import concourse.bass as bass
import concourse.mybir as mybir
import concourse.tile as tile
from concourse import bacc
from concourse.bass_utils import run_bass_kernel_spmd

F32 = mybir.dt.float32
F32R = mybir.dt.float32r

IC = OC = 64
H = W = 64
K = 3
PH = H + 2          # vertically padded height 66
PW = W + 1          # one shared zero column per row (left pad; also serves
                    # as the right pad of the previous row when a kx=2 view
                    # reads contiguously across the row boundary)
PSZ = PW * PH       # 4290
ALLOC = PSZ + 14    # slack so the last kx=2 view's 520-elem slice stays in range
HWPIX = H * W       # 4096
CHUNK = 512         # output pixels per matmul (one PSUM bank)
NCH = HWPIX // CHUNK  # 8 chunks -> 4 chunk-pairs
RPC = CHUNK // W    # 8 image rows per chunk

OFFS = [(ky, kx) for ky in range(K) for kx in range(K)]

MODE = "f32r"       # "f32r" | "f32"


def _build(mode: str = MODE) -> bacc.Bacc:
    mm_dt = F32R if mode == "f32r" else F32
    nc = bacc.Bacc("TRN2", target_bir_lowering=False, debug=False)

    x = nc.dram_tensor("x", [IC, H, W], F32, kind="ExternalInput").ap()
    # host-pretransposed weights: wt[ic, (ky*3+kx)*64 + oc] = Wt[oc, ic, ky, kx]
    wt = nc.dram_tensor("wt", [IC, K * K * OC], F32, kind="ExternalInput").ap()
    zeros = nc.dram_tensor("zeros", [128, 96], F32, kind="ExternalInput").ap()
    y = nc.dram_tensor("y", [OC, HWPIX], F32, kind="ExternalOutput").ap()

    x_src = x.rearrange("c h w -> c (h w)")

    with tile.TileContext(nc) as tc:
        with (
            tc.tile_pool(name="xpad", bufs=1) as xpad_pool,
            tc.tile_pool(name="wsb", bufs=1) as wsb_pool,
            tc.tile_pool(name="osb", bufs=3) as osb_pool,
            tc.tile_pool(name="psum", bufs=4, space="PSUM") as psum_pool,
        ):
            # --- weights: [128, 576]; both halves hold the same data so
            # lhsT.base_partition matches the row strip.
            wsb = wsb_pool.tile([128, K * K * OC], mm_dt)
            for s in (0, 1):
                nc.sync.dma_start(wsb[64 * s:64 * s + 64, :], wt.bitcast(mm_dt))

            # --- padded image (65-wide rows) duplicated into both halves.
            xpad = xpad_pool.tile([128, ALLOC], mm_dt)
            xr = xpad[:, :PSZ].rearrange("p (a b) -> p a b", b=PW)
            zsrc = zeros.bitcast(mm_dt)
            # zero: top pad row, bottom pad row + slack, shared pad column
            nc.sync.dma_start(xpad[:, 0:PW], zsrc[:, :PW])
            nc.sync.dma_start(xpad[:, (PH - 1) * PW:], zsrc[:, :PW + 14])
            nc.sync.dma_start(
                xr[:, 1:PH - 1, 0:1],
                zsrc[:, :H].rearrange("p (a b) -> p a b", b=1),
            )
            # image rows -> rows 1..64, cols 1..64 (one DMA per half,
            # on different HWDGE engines so the queues run in parallel)
            for s in (0, 1):
                nc.sync.dma_start(
                    xr[64 * s:64 * s + 64, 1:PH - 1, 1:PW],
                    x_src.bitcast(mm_dt),
                )

            # --- conv: 4 chunk-pairs; row strip s handles chunk 2q+s with
            # all 9 offsets accumulating into its own PSUM bank.
            for q in range(NCH // 2):
                ps = [
                    psum_pool.tile([64, CHUNK], F32, name=f"ps{s}")
                    for s in (0, 1)
                ]
                for t, (ky, kx) in enumerate(OFFS):
                    for s in (0, 1):
                        c = 2 * q + s
                        o = (c * RPC + ky) * PW + kx
                        rhs = xpad[64 * s:64 * s + 64,
                                   o:o + RPC * PW].rearrange(
                            "p (a b) -> p a b", b=PW)[:, :, :W]
                        lhsT = wsb[64 * s:64 * s + 64,
                                   (ky * K + kx) * OC:(ky * K + kx + 1) * OC]
                        nc.tensor.matmul(
                            ps[s][:, :],
                            lhsT,
                            rhs,
                            start=(t == 0),
                            stop=(t == len(OFFS) - 1),
                            skip_group_check=True,
                        )

                # PSUM -> SBUF -> HBM (both chunks in one 256KB store)
                osb = osb_pool.tile([64, 2 * CHUNK], F32, name="osb")
                for s in (0, 1):
                    nc.vector.tensor_copy(
                        osb[:, s * CHUNK:(s + 1) * CHUNK], ps[s][:, :]
                    )
                nc.sync.dma_start(
                    y[:, 2 * q * CHUNK:(2 * q + 2) * CHUNK], osb[:, :]
                )

    nc.compile()
    return nc


_NC_CACHE: dict[str, bacc.Bacc] = {}
_ZEROS = np.zeros((128, 96), dtype=np.float32)


def kernel(x: np.ndarray, Wt: np.ndarray) -> np.ndarray:
    assert x.shape == (8, IC, H, W) and Wt.shape == (OC, IC, K, K)
    if MODE not in _NC_CACHE:
        _NC_CACHE[MODE] = _build(MODE)
    nc = _NC_CACHE[MODE]

    # wt[ic, (ky*3+kx)*64 + oc]
    wt_t = np.ascontiguousarray(
        Wt.astype(np.float32).transpose(1, 2, 3, 0).reshape(IC, K * K * OC)
    )
    in_maps = [
        {
            "x": np.ascontiguousarray(x[b], dtype=np.float32),
            "wt": wt_t,
            "zeros": _ZEROS,
        }
        for b in range(8)
    ]
    global _last_in_maps
    _last_in_maps = in_maps
    res = run_bass_kernel_spmd(nc, in_maps, core_ids=list(range(8)))
    out = np.stack([r["y"].reshape(OC, H, W) for r in res.results])
    return out.astype(np.float32)


_last_in_maps: list[dict[str, np.ndarray]] = []



# revision 9
# speedup vs baseline: 2.7710x; 2.7710x over previous
"""Trainium2 Bass kernel for nn_KeyedConv2d: 3x3 SAME conv, stride 1.

x: [8, 64, 64, 64] (NCHW), Wt: [64, 64, 3, 3] (OIHW) -> out [8, 64, 64, 64].

Sharding: data-parallel over batch, one image per NeuronCore (8 cores).

Per-core algorithm (v2):
- The 9 kernel offsets are covered by 6 matmuls per 512-pixel output chunk:
  3 "paired" matmuls with 128-deep contraction (offsets (ky,0)+(ky,1) fused:
  SBUF partitions 0-63 hold the padded image, partitions 64-127 hold the
  same image shifted one column, so one matmul contracts both offsets), plus
  3 single matmuls (ky,2) with 64-deep contraction.  Matmul cost on TRN2 is
  proportional to output free size only, so pairing halves PE time for the
  paired offsets.
- ~13 warmup matmuls on a zeroed tile keep the PE busy from ~0.3us so the
  pstate ramp reaches full clock (2.4 GHz) before the real matmuls' deps
  fire; instructions are costed at dep-ready time, so all real matmuls run
  at 213ns instead of 788ns.
- Inputs stream as 8 per-strip HBM->SBUF DMAs (contiguous >=512B elements,
  partition_broadcast duplicates the image into both partition halves);
  Activation/Pool engines convert+copy f32 staging into bf16 padded strip
  tiles (engines cannot cross partitions, hence the broadcast DMA).
- bf16 matmuls, f32 PSUM accumulation, direct PSUM->HBM stores (one PSUM
  bank per chunk, no eviction copy).
"""
import numpy as np
import ml_dtypes

import concourse.bass as bass
import concourse.mybir as mybir
import concourse.tile as tile
from concourse import bacc
from concourse.bass_utils import run_bass_kernel_spmd

F32 = mybir.dt.float32
BF16 = mybir.dt.bfloat16

IC = OC = 64
H = W = 64
K = 3
HWPIX = H * W        # 4096
CHUNK = 512          # output pixels per PSUM bank
NCH = HWPIX // CHUNK  # 8 chunks
RPC = CHUNK // W     # 8 image rows per chunk
PW = W + 1           # padded row width (left zero col; col 64 of the last
                     # window doubles as right pad via row contiguity)
SROWS = RPC + 2      # padded rows per strip incl halo (10)
SLEN = SROWS * PW + 2  # 652: singles' last 8x65 window ends at elem 651

WARMN = 13           # warmup matmuls (tuned so warmup span ~ first chunk ready)

MODE = "bf16"


def _build(mode: str = MODE) -> bacc.Bacc:
    nc = bacc.Bacc("TRN2", target_bir_lowering=False, debug=False)

    x = nc.dram_tensor("x", [IC, H, W], F32, kind="ExternalInput").ap()
    # host-prepacked weights [128, 384] bf16:
    #   wt[ic,       ky*64+oc] = Wt[oc, ic, ky, 0]
    #   wt[64+ic,    ky*64+oc] = Wt[oc, ic, ky, 1]
    #   wt[ic, (3+ky)*64+oc]   = Wt[oc, ic, ky, 2]
    wt = nc.dram_tensor("wt", [128, 2 * K * OC], BF16, kind="ExternalInput").ap()
    y = nc.dram_tensor("y", [OC, HWPIX], F32, kind="ExternalOutput").ap()

    with tile.TileContext(nc) as tc:
        with (
            tc.tile_pool(name="wsb", bufs=1) as wsb_pool,
            tc.tile_pool(name="warm", bufs=1) as warm_pool,
            tc.tile_pool(name="piece", bufs=1) as piece_pool,
            tc.tile_pool(name="xs", bufs=1) as xs_pool,
            tc.tile_pool(name="psum", bufs=1, space="PSUM") as psum_pool,
            tc.tile_pool(name="osb", bufs=3) as osb_pool,
        ):
            # --- warmup source (DVE memset, ready ~0.2us)
            warm = warm_pool.tile([64, 256], BF16)
            nc.vector.memset(warm[:, :], 0.0)

            # --- weights (one 128-partition DMA, no small-elem penalty)
            wsb = wsb_pool.tile([128, 2 * K * OC], BF16)
            nc.sync.dma_start(wsb[:, :], wt)

            # --- warmup matmuls: keep PE busy until first real chunk is ready
            # shares the 8th PSUM bank with chunk 7 (only 8 banks exist); the
            # pool inserts the WAW dep, and both are PE-engine so it is free
            wps = psum_pool.tile([64, CHUNK], F32, name="ps7")
            for _ in range(WARMN):
                nc.tensor.matmul(
                    wps[:, 0:256], warm[:, 0:64], warm[:, 0:256],
                    start=True, stop=True, skip_group_check=True,
                )

            # --- staging pieces: strip k needs image rows 8k-1 .. 8k+8
            pieces = []
            prows = []
            for k in range(NCH):
                r0 = max(8 * k - 1, 0)
                r1 = min(8 * k + 8, H - 1)
                nr = r1 - r0 + 1
                pc = piece_pool.tile([128, nr * W], F32, name=f"pc{k}")
                src = x[:, r0:r1 + 1, :].partition_broadcast(2)
                nc.sync.dma_start(pc[:, :], src)
                pieces.append(pc)
                prows.append((r0, nr))

            # --- strip tiles + zero slivers (DVE) + pad copies (Act top,
            #     Pool bottom, f32->bf16)
            xss = []
            for k in range(NCH):
                xs = xs_pool.tile([128, SLEN], BF16, name=f"xs{k}")
                xss.append(xs)
                # top half left-pad column (rows 0..9 plus the trailing
                # element 650 = "row 10 col 0" read by the ky=2 single)
                nc.vector.memset(
                    xs[0:64, 0:SROWS * PW].rearrange("p (a b) -> p a b", b=PW)[:, :, 0:1],
                    0.0,
                )
                nc.vector.memset(xs[0:64, SROWS * PW:SROWS * PW + 1], 0.0)
                if k == 0:
                    nc.vector.memset(xs[:, 0:PW], 0.0)          # pad row 0
                if k == NCH - 1:
                    nc.vector.memset(xs[:, 9 * PW:9 * PW + PW], 0.0)  # pad row 65

            for k in range(NCH):
                pc = pieces[k]
                xs = xss[k]
                r0, nr = prows[k]
                # strip row r holds padded row 8k+r; img row 8k+r-1
                rlo = 1 if k == 0 else 0
                rhi = 9 if k < NCH - 1 else 8
                nrows = rhi - rlo + 1
                assert nrows == nr
                src = pc[:, :].rearrange("p (a b) -> p a b", b=W)
                dst = xs[:, rlo * PW:(rhi + 1) * PW].rearrange(
                    "p (a b) -> p a b", b=PW)
                # top: cols 1..64 <- img cols 0..63
                nc.scalar.copy(dst[0:64, :, 1:1 + W], src[0:64, :, :])
                # bottom: cols 0..63 <- img cols 0..63 (one col left-shifted)
                nc.gpsimd.tensor_copy(dst[64:128, :, 0:W], src[64:128, :, :])

            # --- conv: per chunk, 3 paired + 3 single matmuls into one bank
            osb = None
            for k in range(NCH):
                xs = xss[k]
                ps = psum_pool.tile([64, CHUNK], F32, name=f"ps{k}")
                for t, ky in enumerate(range(K)):
                    rhs = xs[:, ky * PW:(ky + 8) * PW].rearrange(
                        "p (a b) -> p a b", b=PW)[:, :, 0:W]
                    nc.tensor.matmul(
                        ps[:, :], wsb[:, ky * OC:(ky + 1) * OC], rhs,
                        start=(t == 0), stop=False, skip_group_check=True,
                    )
                for t, ky in enumerate(range(K)):
                    o = ky * PW + 2
                    rhs = xs[0:64, o:o + 8 * PW].rearrange(
                        "p (a b) -> p a b", b=PW)[:, :, 0:W]
                    nc.tensor.matmul(
                        ps[:, :], wsb[0:64, (3 + ky) * OC:(4 + ky) * OC], rhs,
                        start=False, stop=(t == K - 1), skip_group_check=True,
                    )
                # PSUM -> SBUF (DVE) -> HBM (one store per 2 chunks)
                if k % 2 == 0:
                    osb = osb_pool.tile([64, 2 * CHUNK], F32, name="osb")
                nc.vector.tensor_copy(
                    osb[:, (k % 2) * CHUNK:(k % 2 + 1) * CHUNK], ps[:, :]
                )
                if k % 2 == 1:
                    nc.sync.dma_start(
                        y[:, (k - 1) * CHUNK:(k + 1) * CHUNK], osb[:, :]
                    )

    nc.compile()
    return nc


_NC_CACHE: dict[str, bacc.Bacc] = {}


def _pack_weights(Wt: np.ndarray) -> np.ndarray:
    Wf = Wt.astype(np.float32)
    wsb = np.zeros((128, 2 * K * OC), dtype=np.float32)
    # [oc, ic, ky, kx] -> [ic, ky, oc]
    wsb[0:64, 0:K * OC] = Wf[:, :, :, 0].transpose(1, 2, 0).reshape(IC, K * OC)
    wsb[64:128, 0:K * OC] = Wf[:, :, :, 1].transpose(1, 2, 0).reshape(IC, K * OC)
    wsb[0:64, K * OC:2 * K * OC] = (
        Wf[:, :, :, 2].transpose(1, 2, 0).reshape(IC, K * OC)
    )
    return wsb.astype(ml_dtypes.bfloat16)


def kernel(x: np.ndarray, Wt: np.ndarray) -> np.ndarray:
    assert x.shape == (8, IC, H, W) and Wt.shape == (OC, IC, K, K)
    if MODE not in _NC_CACHE:
        _NC_CACHE[MODE] = _build(MODE)
    nc = _NC_CACHE[MODE]

    wt_t = _pack_weights(Wt)
    in_maps = [
        {
            "x": np.ascontiguousarray(x[b], dtype=np.float32),
            "wt": wt_t,
        }
        for b in range(8)
    ]
    global _last_in_maps
    _last_in_maps = in_maps
    res = run_bass_kernel_spmd(nc, in_maps, core_ids=list(range(8)))
    out = np.stack([r["y"].reshape(OC, H, W) for r in res.results])
    return out.astype(np.float32)


_last_in_maps: list[dict[str, np.ndarray]] = []


# revision 11
# speedup vs baseline: 2.8673x; 1.0347x over previous
"""Trainium2 Bass kernel for nn_KeyedConv2d: 3x3 SAME conv, stride 1.

x: [8, 64, 64, 64] (NCHW), Wt: [64, 64, 3, 3] (OIHW) -> out [8, 64, 64, 64].

Sharding: data-parallel over batch, one image per NeuronCore (8 cores).

Per-core algorithm (v2):
- The 9 kernel offsets are covered by 6 matmuls per 512-pixel output chunk:
  3 "paired" matmuls with 128-deep contraction (offsets (ky,0)+(ky,1) fused:
  SBUF partitions 0-63 hold the padded image, partitions 64-127 hold the
  same image shifted one column, so one matmul contracts both offsets), plus
  3 single matmuls (ky,2) with 64-deep contraction.  Matmul cost on TRN2 is
  proportional to output free size only, so pairing halves PE time for the
  paired offsets.
- ~13 warmup matmuls on a zeroed tile keep the PE busy from ~0.3us so the
  pstate ramp reaches full clock (2.4 GHz) before the real matmuls' deps
  fire; instructions are costed at dep-ready time, so all real matmuls run
  at 213ns instead of 788ns.
- Inputs stream as 8 per-strip HBM->SBUF DMAs (contiguous >=512B elements,
  partition_broadcast duplicates the image into both partition halves);
  Activation/Pool engines convert+copy f32 staging into bf16 padded strip
  tiles (engines cannot cross partitions, hence the broadcast DMA).
- bf16 matmuls, f32 PSUM accumulation, direct PSUM->HBM stores (one PSUM
  bank per chunk, no eviction copy).
"""
import numpy as np
import ml_dtypes

import concourse.bass as bass
import concourse.mybir as mybir
import concourse.tile as tile
from concourse import bacc
from concourse.bass_utils import run_bass_kernel_spmd

F32 = mybir.dt.float32
BF16 = mybir.dt.bfloat16

IC = OC = 64
H = W = 64
K = 3
HWPIX = H * W        # 4096
CHUNK = 512          # output pixels per PSUM bank
NCH = HWPIX // CHUNK  # 8 chunks
RPC = CHUNK // W     # 8 image rows per chunk
PW = W + 1           # padded row width (left zero col; col 64 of the last
                     # window doubles as right pad via row contiguity)
SROWS = RPC + 2      # padded rows per strip incl halo (10)
SLEN = SROWS * PW + 2  # 652: singles' last 8x65 window ends at elem 651

WARMN = 19           # warmup matmuls (tuned so warmup span ~ first chunk ready)

MODE = "bf16"


def _build(mode: str = MODE) -> bacc.Bacc:
    nc = bacc.Bacc("TRN2", target_bir_lowering=False, debug=False)

    x = nc.dram_tensor("x", [IC, H, W], F32, kind="ExternalInput").ap()
    # host-prepacked weights [128, 384] bf16:
    #   wt[ic,       ky*64+oc] = Wt[oc, ic, ky, 0]
    #   wt[64+ic,    ky*64+oc] = Wt[oc, ic, ky, 1]
    #   wt[ic, (3+ky)*64+oc]   = Wt[oc, ic, ky, 2]
    wt = nc.dram_tensor("wt", [128, 2 * K * OC], BF16, kind="ExternalInput").ap()
    y = nc.dram_tensor("y", [OC, HWPIX], BF16, kind="ExternalOutput").ap()

    with tile.TileContext(nc) as tc:
        with (
            tc.tile_pool(name="wsb", bufs=1) as wsb_pool,
            tc.tile_pool(name="warm", bufs=1) as warm_pool,
            tc.tile_pool(name="piece", bufs=1) as piece_pool,
            tc.tile_pool(name="xs", bufs=1) as xs_pool,
            tc.tile_pool(name="psum", bufs=1, space="PSUM") as psum_pool,
            tc.tile_pool(name="osb", bufs=3) as osb_pool,
        ):
            # --- warmup source (DVE memset, ready ~0.2us)
            warm = warm_pool.tile([64, 256], BF16)
            nc.vector.memset(warm[:, :], 0.0)

            # --- warmup matmuls: keep PE busy until first real chunk is
            # ready.  Shares PSUM bank (tag) with chunk 0; all deps are
            # PE-program-order so the sharing is free.
            wps = psum_pool.tile([64, CHUNK], F32, name="ps0")
            for _ in range(WARMN):
                nc.tensor.matmul(
                    wps[:, 0:128], warm[:, 0:64], warm[:, 0:128],
                    start=True, stop=True, skip_group_check=True,
                )

            # --- staging pieces: strip k needs image rows 8k-1 .. 8k+8.
            # Piece 0 is split in two and issued first so strip 0's pad
            # copies (and hence the first matmuls) start as early as
            # possible; the weights DMA slots between them.
            pieces = {}
            prows = {}
            for key, r0, r1 in (("0a", 0, 3), ("0b", 4, 8)):
                nr = r1 - r0 + 1
                pc = piece_pool.tile([128, nr * W], F32, name=f"pc{key}")
                nc.sync.dma_start(
                    pc[:, :], x[:, r0:r1 + 1, :].partition_broadcast(2)
                )
                pieces[key] = pc
                prows[key] = (r0, nr)

            # --- weights (one 128-partition DMA, no small-elem penalty)
            wsb = wsb_pool.tile([128, 2 * K * OC], BF16)
            nc.sync.dma_start(wsb[:, :], wt)

            for k in range(1, NCH):
                r0 = max(8 * k - 1, 0)
                r1 = min(8 * k + 8, H - 1)
                nr = r1 - r0 + 1
                pc = piece_pool.tile([128, nr * W], F32, name=f"pc{k}")
                nc.sync.dma_start(
                    pc[:, :], x[:, r0:r1 + 1, :].partition_broadcast(2)
                )
                pieces[k] = pc
                prows[k] = (r0, nr)

            # --- strip tiles + zero slivers (DVE) + pad copies (Act top,
            #     Pool bottom, f32->bf16)
            xss = []
            for k in range(NCH):
                xs = xs_pool.tile([128, SLEN], BF16, name=f"xs{k}")
                xss.append(xs)
                # top half left-pad column (rows 0..9 plus the trailing
                # element 650 = "row 10 col 0" read by the ky=2 single)
                nc.vector.memset(
                    xs[0:64, 0:SROWS * PW].rearrange("p (a b) -> p a b", b=PW)[:, :, 0:1],
                    0.0,
                )
                nc.vector.memset(xs[0:64, SROWS * PW:SROWS * PW + 1], 0.0)
                if k == 0:
                    nc.vector.memset(xs[:, 0:PW], 0.0)          # pad row 0
                if k == NCH - 1:
                    nc.vector.memset(xs[:, 9 * PW:9 * PW + PW], 0.0)  # pad row 65

            # (strip0 is fed by the two piece-0 halves: strip rows 1-4, 5-9)
            copy_jobs = [(0, "0a", 1, 4), (0, "0b", 5, 9)]
            copy_jobs += [
                (k, k, 0, 9 if k < NCH - 1 else 8) for k in range(1, NCH)
            ]
            for k, key, rlo, rhi in copy_jobs:
                pc = pieces[key]
                xs = xss[k]
                r0, nr = prows[key]
                # strip row r holds padded row 8k+r = img row 8k+r-1
                assert rhi - rlo + 1 == nr and 8 * k + rlo - 1 == r0
                src = pc[:, :].rearrange("p (a b) -> p a b", b=W)
                dst = xs[:, rlo * PW:(rhi + 1) * PW].rearrange(
                    "p (a b) -> p a b", b=PW)
                # top: cols 1..64 <- img cols 0..63
                nc.scalar.copy(dst[0:64, :, 1:1 + W], src[0:64, :, :])
                # bottom: cols 0..63 <- img cols 0..63 (one col left-shifted)
                nc.gpsimd.tensor_copy(dst[64:128, :, 0:W], src[64:128, :, :])

            # --- conv: per chunk, 3 paired + 3 single matmuls into one
            # PSUM bank.  Work items: (strip k, first strip row, npix, psum
            # tag).  Chunk 7 is split in two 256-px halves so the final
            # eviction+store tail after the last matmul is short.  The two
            # halves recycle the ps0/ps1 bank tags (long since evicted).
            work = [
                (k, 0, CHUNK, f"ps{k}", k % 2 == 1) for k in range(NCH - 1)
            ]
            work += [(7, 0, CHUNK // 2, "ps0", True),
                     (7, 4, CHUNK // 2, "ps1", True)]
            evicted = []
            for k, rbase, npix, tag, flush in work:
                xs = xss[k]
                nrows = npix // W
                ps = psum_pool.tile([64, npix], F32, name=tag)
                for t, ky in enumerate(range(K)):
                    o = (rbase + ky) * PW
                    rhs = xs[:, o:o + nrows * PW].rearrange(
                        "p (a b) -> p a b", b=PW)[:, :, 0:W]
                    nc.tensor.matmul(
                        ps[:, :], wsb[:, ky * OC:(ky + 1) * OC], rhs,
                        start=(t == 0), stop=False, skip_group_check=True,
                    )
                for t, ky in enumerate(range(K)):
                    o = (rbase + ky) * PW + 2
                    rhs = xs[0:64, o:o + nrows * PW].rearrange(
                        "p (a b) -> p a b", b=PW)[:, :, 0:W]
                    nc.tensor.matmul(
                        ps[:, :], wsb[0:64, (3 + ky) * OC:(4 + ky) * OC], rhs,
                        start=False, stop=(t == K - 1), skip_group_check=True,
                    )
                evicted.append((k * CHUNK + rbase * W, npix, ps))
                # PSUM -> SBUF bf16 (DVE) -> HBM; batch stores so the final
                # one is the lone 256-px half-chunk
                if flush:
                    base = evicted[0][0]
                    tot = sum(n for _, n, _ in evicted)
                    osb = osb_pool.tile([64, tot], BF16, name="osb")
                    off = 0
                    for _, n, p in evicted:
                        nc.vector.tensor_copy(osb[:, off:off + n], p[:, :])
                        off += n
                    nc.sync.dma_start(y[:, base:base + tot], osb[:, :])
                    evicted = []

    nc.compile()
    return nc


_NC_CACHE: dict[str, bacc.Bacc] = {}


def _pack_weights(Wt: np.ndarray) -> np.ndarray:
    Wf = Wt.astype(np.float32)
    wsb = np.zeros((128, 2 * K * OC), dtype=np.float32)
    # [oc, ic, ky, kx] -> [ic, ky, oc]
    wsb[0:64, 0:K * OC] = Wf[:, :, :, 0].transpose(1, 2, 0).reshape(IC, K * OC)
    wsb[64:128, 0:K * OC] = Wf[:, :, :, 1].transpose(1, 2, 0).reshape(IC, K * OC)
    wsb[0:64, K * OC:2 * K * OC] = (
        Wf[:, :, :, 2].transpose(1, 2, 0).reshape(IC, K * OC)
    )
    return wsb.astype(ml_dtypes.bfloat16)


def kernel(x: np.ndarray, Wt: np.ndarray) -> np.ndarray:
    assert x.shape == (8, IC, H, W) and Wt.shape == (OC, IC, K, K)
    if MODE not in _NC_CACHE:
        _NC_CACHE[MODE] = _build(MODE)
    nc = _NC_CACHE[MODE]

    wt_t = _pack_weights(Wt)
    in_maps = [
        {
            "x": np.ascontiguousarray(x[b], dtype=np.float32),
            "wt": wt_t,
        }
        for b in range(8)
    ]
    global _last_in_maps
    _last_in_maps = in_maps
    res = run_bass_kernel_spmd(nc, in_maps, core_ids=list(range(8)))
    out = np.stack([r["y"].reshape(OC, H, W) for r in res.results])
    return out.astype(np.float32)


_last_in_maps: list[dict[str, np.ndarray]] = []


# revision 16
# speedup vs baseline: 2.9690x; 1.0355x over previous
"""Trainium2 Bass kernel for nn_KeyedConv2d: 3x3 SAME conv, stride 1.

x: [8, 64, 64, 64] (NCHW), Wt: [64, 64, 3, 3] (OIHW) -> out [8, 64, 64, 64].

Sharding: data-parallel over batch, one image per NeuronCore (8 cores).

Per-core algorithm (v2):
- The 9 kernel offsets are covered by 6 matmuls per 512-pixel output chunk:
  3 "paired" matmuls with 128-deep contraction (offsets (ky,0)+(ky,1) fused:
  SBUF partitions 0-63 hold the padded image, partitions 64-127 hold the
  same image shifted one column, so one matmul contracts both offsets), plus
  3 single matmuls (ky,2) with 64-deep contraction.  Matmul cost on TRN2 is
  proportional to output free size only, so pairing halves PE time for the
  paired offsets.
- ~13 warmup matmuls on a zeroed tile keep the PE busy from ~0.3us so the
  pstate ramp reaches full clock (2.4 GHz) before the real matmuls' deps
  fire; instructions are costed at dep-ready time, so all real matmuls run
  at 213ns instead of 788ns.
- Inputs stream as 8 per-strip HBM->SBUF DMAs (contiguous >=512B elements,
  partition_broadcast duplicates the image into both partition halves);
  Activation/Pool engines convert+copy f32 staging into bf16 padded strip
  tiles (engines cannot cross partitions, hence the broadcast DMA).
- bf16 matmuls, f32 PSUM accumulation, direct PSUM->HBM stores (one PSUM
  bank per chunk, no eviction copy).
"""
import numpy as np
import ml_dtypes

import concourse.bass as bass
import concourse.mybir as mybir
import concourse.tile as tile
from concourse import bacc
from concourse.bass_utils import run_bass_kernel_spmd

F32 = mybir.dt.float32
BF16 = mybir.dt.bfloat16

IC = OC = 64
H = W = 64
K = 3
HWPIX = H * W        # 4096
CHUNK = 512          # output pixels per PSUM bank
NCH = HWPIX // CHUNK  # 8 chunks
RPC = CHUNK // W     # 8 image rows per chunk
PW = W + 1           # padded row width (left zero col; col 64 of the last
                     # window doubles as right pad via row contiguity)
SROWS = RPC + 2      # padded rows per strip incl halo (10)
SLEN = SROWS * PW + 2  # 652: singles' last 8x65 window ends at elem 651

WARMN = 19           # warmup matmuls (tuned so warmup span ~ first chunk ready)

MODE = "bf16"


def _build(mode: str = MODE) -> bacc.Bacc:
    nc = bacc.Bacc("TRN2", target_bir_lowering=False, debug=False)

    x = nc.dram_tensor("x", [IC, H, W], F32, kind="ExternalInput").ap()
    # host-prepacked weights [128, 384] bf16:
    #   wt[ic,       ky*64+oc] = Wt[oc, ic, ky, 0]
    #   wt[64+ic,    ky*64+oc] = Wt[oc, ic, ky, 1]
    #   wt[ic, (3+ky)*64+oc]   = Wt[oc, ic, ky, 2]
    wt = nc.dram_tensor("wt", [128, 2 * K * OC], BF16, kind="ExternalInput").ap()
    y = nc.dram_tensor("y", [OC, HWPIX], BF16, kind="ExternalOutput").ap()

    with tile.TileContext(nc) as tc:
        with (
            tc.tile_pool(name="wsb", bufs=1) as wsb_pool,
            tc.tile_pool(name="warm", bufs=1) as warm_pool,
            tc.tile_pool(name="piece", bufs=1) as piece_pool,
            tc.tile_pool(name="xs", bufs=1) as xs_pool,
            tc.tile_pool(name="psum", bufs=1, space="PSUM") as psum_pool,
            tc.tile_pool(name="osb", bufs=6) as osb_pool,
        ):
            # --- warmup source (DVE memset, ready ~0.2us)
            warm = warm_pool.tile([64, 256], BF16)
            nc.vector.memset(warm[:, :], 0.0)

            # --- warmup matmuls: keep PE busy until first real chunk is
            # ready.  Shares PSUM bank (tag) with chunk 0; all deps are
            # PE-program-order so the sharing is free.
            wps = psum_pool.tile([64, CHUNK], F32, name="ps0")
            for _ in range(WARMN):
                nc.tensor.matmul(
                    wps[:, 0:128], warm[:, 0:64], warm[:, 0:128],
                    start=True, stop=True, skip_group_check=True,
                )

            # --- staging pieces: strip k needs image rows 8k-1 .. 8k+8.
            # Piece 0 is split in two and issued first so strip 0's pad
            # copies (and hence the first matmuls) start as early as
            # possible; the weights DMA slots between them.
            pieces = {}
            prows = {}
            for key, r0, r1 in (("0a", 0, 4), ("0b", 5, 8)):
                nr = r1 - r0 + 1
                pc = piece_pool.tile([128, nr * W], F32, name=f"pc{key}")
                nc.sync.dma_start(
                    pc[:, :], x[:, r0:r1 + 1, :].partition_broadcast(2)
                )
                pieces[key] = pc
                prows[key] = (r0, nr)

            # --- weights via SWDGE (gpsimd): stays off the serial HWDGE
            # chain so the piece DMAs pipeline back-to-back
            wsb = wsb_pool.tile([128, 2 * K * OC], BF16)
            nc.gpsimd.dma_start(wsb[:, :], wt)

            for k in range(1, NCH):
                r0 = max(8 * k - 1, 0)
                r1 = min(8 * k + 8, H - 1)
                nr = r1 - r0 + 1
                pc = piece_pool.tile([128, nr * W], F32, name=f"pc{k}")
                nc.sync.dma_start(
                    pc[:, :], x[:, r0:r1 + 1, :].partition_broadcast(2)
                )
                pieces[k] = pc
                prows[k] = (r0, nr)

            # --- strip tiles + zero slivers (DVE) + pad copies (Act top,
            #     Pool bottom, f32->bf16)
            xss = []
            for k in range(NCH):
                xs = xs_pool.tile([128, SLEN], BF16, name=f"xs{k}")
                xss.append(xs)
                # top half left-pad column (rows 0..9 plus the trailing
                # element 650 = "row 10 col 0" read by the ky=2 single)
                nc.vector.memset(
                    xs[0:64, 0:SROWS * PW].rearrange("p (a b) -> p a b", b=PW)[:, :, 0:1],
                    0.0,
                )
                nc.vector.memset(xs[0:64, SROWS * PW:SROWS * PW + 1], 0.0)
                if k == 0:
                    nc.vector.memset(xs[:, 0:PW], 0.0)          # pad row 0
                if k == NCH - 1:
                    nc.vector.memset(xs[:, 9 * PW:9 * PW + PW], 0.0)  # pad row 65

            # (strip0 is fed by the two piece-0 halves: strip rows 1-5, 6-9)
            copy_jobs = [(0, "0a", 1, 5), (0, "0b", 6, 9)]
            copy_jobs += [
                (k, k, 0, 9 if k < NCH - 1 else 8) for k in range(1, NCH)
            ]
            for k, key, rlo, rhi in copy_jobs:
                pc = pieces[key]
                xs = xss[k]
                r0, nr = prows[key]
                # strip row r holds padded row 8k+r = img row 8k+r-1
                assert rhi - rlo + 1 == nr and 8 * k + rlo - 1 == r0
                src = pc[:, :].rearrange("p (a b) -> p a b", b=W)
                dst = xs[:, rlo * PW:(rhi + 1) * PW].rearrange(
                    "p (a b) -> p a b", b=PW)
                # top: cols 1..64 <- img cols 0..63
                nc.scalar.copy(dst[0:64, :, 1:1 + W], src[0:64, :, :])
                # bottom: cols 0..63 <- img cols 0..63 (one col left-shifted).
                # Strips 0-1 gate the PE pipeline start: DVE (idle then) beats
                # Pool's 1/0.6-efficiency copies on the critical path.
                beng = nc.vector if k <= 1 else nc.gpsimd
                beng.tensor_copy(dst[64:128, :, 0:W], src[64:128, :, :])

            # --- conv: per chunk, 3 paired + 3 single matmuls into one
            # PSUM bank.  Work items: (strip k, first strip row, npix, psum
            # tag).  Chunk 7 is split in two 256-px halves so the final
            # eviction+store tail after the last matmul is short.  The two
            # halves recycle the ps0/ps1 bank tags (long since evicted).
            work = [
                (k, 0, CHUNK, f"ps{k}", k % 2 == 1 or k == 6)
                for k in range(NCH - 1)
            ]
            work += [(7, 0, 6 * W, "ps0", False), (7, 6, 2 * W, "ps1", True)]
            evicted = []
            for k, rbase, npix, tag, flush in work:
                xs = xss[k]
                nrows = npix // W
                ps = psum_pool.tile([64, npix], F32, name=tag)
                for t, ky in enumerate(range(K)):
                    o = (rbase + ky) * PW
                    rhs = xs[:, o:o + nrows * PW].rearrange(
                        "p (a b) -> p a b", b=PW)[:, :, 0:W]
                    nc.tensor.matmul(
                        ps[:, :], wsb[:, ky * OC:(ky + 1) * OC], rhs,
                        start=(t == 0), stop=False, skip_group_check=True,
                    )
                for t, ky in enumerate(range(K)):
                    o = (rbase + ky) * PW + 2
                    rhs = xs[0:64, o:o + nrows * PW].rearrange(
                        "p (a b) -> p a b", b=PW)[:, :, 0:W]
                    nc.tensor.matmul(
                        ps[:, :], wsb[0:64, (3 + ky) * OC:(4 + ky) * OC], rhs,
                        start=False, stop=(t == K - 1), skip_group_check=True,
                    )
                # PSUM -> SBUF bf16 (DVE) eagerly per chunk; batched HBM
                # store at flush points (last flush = lone 128-px half-chunk
                # eviction, keeping the post-last-matmul tail short)
                if not evicted:
                    gbase = k * CHUNK + rbase * W
                    osb = osb_pool.tile([64, 2 * CHUNK], BF16, name="osb")
                # the 6-row half-chunk's eviction overlaps the final
                # matmuls on Activation so only the 128-px eviction trails
                if tag == "ps0" and k == 7:
                    nc.scalar.copy(
                        osb[:, sum(evicted):sum(evicted) + npix], ps[:, :]
                    )
                else:
                    nc.vector.tensor_copy(
                        osb[:, sum(evicted):sum(evicted) + npix], ps[:, :]
                    )
                evicted.append(npix)
                if flush:
                    tot = sum(evicted)
                    nc.sync.dma_start(
                        y[:, gbase:gbase + tot], osb[:, 0:tot]
                    )
                    evicted = []

    nc.compile()
    return nc


_NC_CACHE: dict[str, bacc.Bacc] = {}


def _pack_weights(Wt: np.ndarray) -> np.ndarray:
    Wf = Wt.astype(np.float32)
    wsb = np.zeros((128, 2 * K * OC), dtype=np.float32)
    # [oc, ic, ky, kx] -> [ic, ky, oc]
    wsb[0:64, 0:K * OC] = Wf[:, :, :, 0].transpose(1, 2, 0).reshape(IC, K * OC)
    wsb[64:128, 0:K * OC] = Wf[:, :, :, 1].transpose(1, 2, 0).reshape(IC, K * OC)
    wsb[0:64, K * OC:2 * K * OC] = (
        Wf[:, :, :, 2].transpose(1, 2, 0).reshape(IC, K * OC)
    )
    return wsb.astype(ml_dtypes.bfloat16)


def kernel(x: np.ndarray, Wt: np.ndarray) -> np.ndarray:
    assert x.shape == (8, IC, H, W) and Wt.shape == (OC, IC, K, K)
    if MODE not in _NC_CACHE:
        _NC_CACHE[MODE] = _build(MODE)
    nc = _NC_CACHE[MODE]

    wt_t = _pack_weights(Wt)
    in_maps = [
        {
            "x": np.ascontiguousarray(x[b], dtype=np.float32),
            "wt": wt_t,
        }
        for b in range(8)
    ]
    global _last_in_maps
    _last_in_maps = in_maps
    res = run_bass_kernel_spmd(nc, in_maps, core_ids=list(range(8)))
    out = np.stack([r["y"].reshape(OC, H, W) for r in res.results])
    return out.astype(np.float32)


_last_in_maps: list[dict[str, np.ndarray]] = []


# revision 20
# speedup vs baseline: 3.0788x; 1.0370x over previous
"""Trainium2 Bass kernel for nn_KeyedConv2d: 3x3 SAME conv, stride 1.

x: [8, 64, 64, 64] (NCHW), Wt: [64, 64, 3, 3] (OIHW) -> out [8, 64, 64, 64].

Sharding: data-parallel over batch, one image per NeuronCore (8 cores).

Per-core algorithm (v2):
- The 9 kernel offsets are covered by 6 matmuls per 512-pixel output chunk:
  3 "paired" matmuls with 128-deep contraction (offsets (ky,0)+(ky,1) fused:
  SBUF partitions 0-63 hold the padded image, partitions 64-127 hold the
  same image shifted one column, so one matmul contracts both offsets), plus
  3 single matmuls (ky,2) with 64-deep contraction.  Matmul cost on TRN2 is
  proportional to output free size only, so pairing halves PE time for the
  paired offsets.
- ~13 warmup matmuls on a zeroed tile keep the PE busy from ~0.3us so the
  pstate ramp reaches full clock (2.4 GHz) before the real matmuls' deps
  fire; instructions are costed at dep-ready time, so all real matmuls run
  at 213ns instead of 788ns.
- Inputs stream as 8 per-strip HBM->SBUF DMAs (contiguous >=512B elements,
  partition_broadcast duplicates the image into both partition halves);
  Activation/Pool engines convert+copy f32 staging into bf16 padded strip
  tiles (engines cannot cross partitions, hence the broadcast DMA).
- bf16 matmuls, f32 PSUM accumulation, direct PSUM->HBM stores (one PSUM
  bank per chunk, no eviction copy).
"""
import numpy as np
import ml_dtypes

import concourse.bass as bass
import concourse.mybir as mybir
import concourse.tile as tile
from concourse import bacc
from concourse.bass_utils import run_bass_kernel_spmd

F32 = mybir.dt.float32
BF16 = mybir.dt.bfloat16

IC = OC = 64
H = W = 64
K = 3
HWPIX = H * W        # 4096
CHUNK = 512          # output pixels per PSUM bank
NCH = HWPIX // CHUNK  # 8 chunks
RPC = CHUNK // W     # 8 image rows per chunk
PW = W + 1           # padded row width (left zero col; col 64 of the last
                     # window doubles as right pad via row contiguity)
SROWS = RPC + 2      # padded rows per strip incl halo (10)
SLEN = SROWS * PW + 2  # 652: singles' last 8x65 window ends at elem 651
RB = 656             # region B base: holds (A-top shifted 2, A-bot shifted
                     # 66) so one matmul pairs offsets (ky,2)+(ky+1,2)
RBLEN = 8 * PW       # 520
TLEN = RB + RBLEN    # strip tile total width

WARMN = 19           # warmup matmuls (tuned so warmup span ~ first chunk ready)

MODE = "bf16"


def _build(mode: str = MODE) -> bacc.Bacc:
    nc = bacc.Bacc("TRN2", target_bir_lowering=False, debug=False)

    # image pre-cast to bf16 on the host: halves the piece-DMA bytes (the
    # matmuls consume bf16 anyway)
    xbf = nc.dram_tensor("xbf", [IC, H, W], BF16, kind="ExternalInput").ap()
    # host-prepacked weights [128, 384] bf16:
    #   wt[ic,       ky*64+oc] = Wt[oc, ic, ky, 0]
    #   wt[64+ic,    ky*64+oc] = Wt[oc, ic, ky, 1]
    #   wt[ic, (3+ky)*64+oc]   = Wt[oc, ic, ky, 2]
    wt = nc.dram_tensor("wt", [128, 5 * OC], BF16, kind="ExternalInput").ap()
    y = nc.dram_tensor("y", [OC, HWPIX], BF16, kind="ExternalOutput").ap()

    with tile.TileContext(nc) as tc:
        with (
            tc.tile_pool(name="wsb", bufs=1) as wsb_pool,
            tc.tile_pool(name="warm", bufs=1) as warm_pool,
            tc.tile_pool(name="piece", bufs=1) as piece_pool,
            tc.tile_pool(name="xs", bufs=1) as xs_pool,
            tc.tile_pool(name="psum", bufs=1, space="PSUM") as psum_pool,
            tc.tile_pool(name="osb", bufs=6) as osb_pool,
        ):
            # --- warmup source (DVE memset, ready ~0.2us)
            warm = warm_pool.tile([64, 256], BF16)
            nc.vector.memset(warm[:, :], 0.0)

            # --- warmup matmuls: keep PE busy until first real chunk is
            # ready.  Shares PSUM bank (tag) with chunk 0; all deps are
            # PE-program-order so the sharing is free.
            wps = psum_pool.tile([64, CHUNK], F32, name="ps0")
            for _ in range(WARMN):
                nc.tensor.matmul(
                    wps[:, 0:128], warm[:, 0:64], warm[:, 0:128],
                    start=True, stop=True, skip_group_check=True,
                )

            # --- staging pieces: strip k needs image rows 8k-1 .. 8k+8.
            # Piece 0 is split in two and issued first so strip 0's pad
            # copies (and hence the first matmuls) start as early as
            # possible; the weights DMA slots between them.
            pieces = {}
            prows = {}
            for key, r0, r1 in (("0a", 0, 6), ("0b", 7, 8)):
                nr = r1 - r0 + 1
                pc = piece_pool.tile([128, nr * W], BF16, name=f"pc{key}")
                nc.sync.dma_start(
                    pc[:, :], xbf[:, r0:r1 + 1, :].partition_broadcast(2)
                )
                pieces[key] = pc
                prows[key] = (r0, nr)

            # --- weights via SWDGE (gpsimd): stays off the serial HWDGE
            # chain so the piece DMAs pipeline back-to-back
            wsb = wsb_pool.tile([128, 5 * OC], BF16)
            nc.gpsimd.dma_start(wsb[:, :], wt)

            for k in range(1, NCH):
                r0 = max(8 * k - 1, 0)
                r1 = min(8 * k + 8, H - 1)
                nr = r1 - r0 + 1
                pc = piece_pool.tile([128, nr * W], BF16, name=f"pc{k}")
                nc.sync.dma_start(
                    pc[:, :], xbf[:, r0:r1 + 1, :].partition_broadcast(2)
                )
                pieces[k] = pc
                prows[k] = (r0, nr)

            # --- strip tiles + zero slivers (DVE) + pad copies (Act top,
            #     Pool bottom, f32->bf16)
            xss = []
            for k in range(NCH):
                xs = xs_pool.tile([128, TLEN], BF16, name=f"xs{k}")
                xss.append(xs)
                # top half left-pad column (rows 0..9 plus the trailing
                # element 650 = "row 10 col 0" read by the ky=2 single)
                nc.vector.memset(
                    xs[0:64, 0:SROWS * PW].rearrange("p (a b) -> p a b", b=PW)[:, :, 0:1],
                    0.0,
                )
                nc.vector.memset(xs[0:64, SROWS * PW:SROWS * PW + 1], 0.0)
                # bottom col 64 (left pad of row r+1 seen from the bottom
                # half): copied into region B, so it must be zero
                nc.vector.memset(
                    xs[64:128, 0:SROWS * PW].rearrange(
                        "p (a b) -> p a b", b=PW)[:, :, W:PW],
                    0.0,
                )
                if k == 0:
                    nc.vector.memset(xs[:, 0:PW], 0.0)          # pad row 0
                if k == NCH - 1:
                    nc.vector.memset(xs[:, 9 * PW:9 * PW + PW], 0.0)  # pad row 65

            # Pad-copy jobs (strip, piece, rows, half, engine).  Strips 0-1
            # gate the PE pipeline start, so their copies are split finely
            # and lean on DVE's 2x 16-bit mode; steady-state strips use
            # Act (tops) and Pool (bottoms) in parallel.
            T, B = 0, 1
            copy_jobs = [
                (0, "0a", 1, 7, T, nc.scalar), (0, "0a", 1, 7, B, nc.vector),
                (0, "0b", 8, 9, T, nc.vector), (0, "0b", 8, 9, B, nc.vector),
                (1, 1, 0, 6, T, nc.scalar), (1, 1, 7, 9, T, nc.vector),
                (1, 1, 0, 9, B, nc.vector),
            ]
            copy_jobs += [
                (k, k, 0, 9 if k < NCH - 1 else 8, h, e)
                for k in range(2, NCH)
                for h, e in ((T, nc.scalar), (B, nc.gpsimd))
            ]
            for k, key, rlo, rhi, half, eng in copy_jobs:
                pc = pieces[key]
                xs = xss[k]
                r0, nr = prows[key]
                # strip row r holds padded row 8k+r = img row 8k+r-1
                off = 8 * k + rlo - 1 - r0
                assert 0 <= off and off + (rhi - rlo) < nr
                src = pc[:, off * W:(off + rhi - rlo + 1) * W].rearrange(
                    "p (a b) -> p a b", b=W)
                dst = xs[:, rlo * PW:(rhi + 1) * PW].rearrange(
                    "p (a b) -> p a b", b=PW)
                if half == T:
                    # top: cols 1..64 <- img cols 0..63
                    if eng is nc.scalar:
                        eng.copy(dst[0:64, :, 1:1 + W], src[0:64, :, :])
                    else:
                        eng.tensor_copy(dst[0:64, :, 1:1 + W], src[0:64, :, :])
                else:
                    # bottom: cols 0..63 <- img (one col left-shifted)
                    eng.tensor_copy(dst[64:128, :, 0:W], src[64:128, :, :])

            # --- region B: B-top = A-top shifted 2 cols, B-bot = A-bot
            # shifted 66 (one row + one col).  Reading region B at row base r
            # then contracts offsets (0,2) [top] and (1,2) [bottom] in one
            # matmul.  DVE 2x-mode copies; A's zero pads come along free.
            # Emitted one strip ahead of the consuming chunk so the DVE
            # wait-queue (FIFO, depth 4) never head-blocks an eviction.
            def emit_region_b(k):
                xs = xss[k]
                nc.vector.tensor_copy(
                    xs[0:64, RB:RB + 8 * PW - 1], xs[0:64, 2:2 + 8 * PW - 1]
                )
                nc.vector.tensor_copy(
                    xs[64:128, RB:RB + 8 * PW - 1],
                    xs[64:128, 66:66 + 8 * PW - 1],
                )

            emit_region_b(0)
            emit_region_b(1)
            next_b = 2

            # --- conv: per chunk, 3 paired + 3 single matmuls into one
            # PSUM bank.  Work items: (strip k, first strip row, npix, psum
            # tag).  Chunk 7 is split in two 256-px halves so the final
            # eviction+store tail after the last matmul is short.  The two
            # halves recycle the ps0/ps1 bank tags (long since evicted).
            work = [
                (k, 0, CHUNK, f"ps{k}", k % 2 == 1 or k == 6)
                for k in range(NCH - 1)
            ]
            work += [(7, 0, 6 * W, "ps0", False), (7, 6, 2 * W, "ps1", True)]
            evicted = []
            for k, rbase, npix, tag, flush in work:
                xs = xss[k]
                if k + 1 >= next_b and k + 1 < NCH:
                    emit_region_b(k + 1)
                    next_b = k + 2
                nrows = npix // W
                ps = psum_pool.tile([64, npix], F32, name=tag)
                for t, ky in enumerate(range(K)):
                    o = (rbase + ky) * PW
                    rhs = xs[:, o:o + nrows * PW].rearrange(
                        "p (a b) -> p a b", b=PW)[:, :, 0:W]
                    nc.tensor.matmul(
                        ps[:, :], wsb[:, ky * OC:(ky + 1) * OC], rhs,
                        start=(t == 0), stop=False, skip_group_check=True,
                    )
                # single (2,2) from region A's top half
                o = (rbase + 2) * PW + 2
                rhs = xs[0:64, o:o + nrows * PW].rearrange(
                    "p (a b) -> p a b", b=PW)[:, :, 0:W]
                nc.tensor.matmul(
                    ps[:, :], wsb[0:64, 4 * OC:5 * OC], rhs,
                    start=False, stop=False, skip_group_check=True,
                )
                # paired (0,2)+(1,2) from region B (last: B copies get the
                # longest overlap window behind the 4 region-A matmuls)
                o = RB + rbase * PW
                rhs = xs[:, o:o + nrows * PW].rearrange(
                    "p (a b) -> p a b", b=PW)[:, :, 0:W]
                nc.tensor.matmul(
                    ps[:, :], wsb[:, 3 * OC:4 * OC], rhs,
                    start=False, stop=True, skip_group_check=True,
                )
                # PSUM -> SBUF bf16 (DVE) eagerly per chunk; batched HBM
                # store at flush points (last flush = lone 128-px half-chunk
                # eviction, keeping the post-last-matmul tail short)
                if not evicted:
                    gbase = k * CHUNK + rbase * W
                    osb = osb_pool.tile([64, 2 * CHUNK], BF16, name="osb")
                # evictions alternate DVE/Act (both also feed pad/region-B
                # copies); the 6-row half-chunk lands on Act so only the
                # final 128-px eviction trails the last matmul
                odst = osb[:, sum(evicted):sum(evicted) + npix]
                if (k % 2 == 1) or (tag == "ps0" and k == 7):
                    nc.scalar.copy(odst, ps[:, :])
                else:
                    nc.vector.tensor_copy(odst, ps[:, :])
                evicted.append(npix)
                if flush:
                    tot = sum(evicted)
                    nc.sync.dma_start(
                        y[:, gbase:gbase + tot], osb[:, 0:tot]
                    )
                    evicted = []

    nc.compile()
    return nc


_NC_CACHE: dict[str, bacc.Bacc] = {}


def _pack_weights(Wt: np.ndarray) -> np.ndarray:
    Wf = Wt.astype(np.float32)
    wsb = np.zeros((128, 5 * OC), dtype=np.float32)
    # blocks 0-2: A-pairs (ky, kx=0) top / (ky, kx=1) bottom
    # [oc, ic, ky, kx] -> [ic, ky, oc]
    wsb[0:64, 0:K * OC] = Wf[:, :, :, 0].transpose(1, 2, 0).reshape(IC, K * OC)
    wsb[64:128, 0:K * OC] = Wf[:, :, :, 1].transpose(1, 2, 0).reshape(IC, K * OC)
    # block 3: B-pair (0,2) top / (1,2) bottom
    wsb[0:64, 3 * OC:4 * OC] = Wf[:, :, 0, 2].T
    wsb[64:128, 3 * OC:4 * OC] = Wf[:, :, 1, 2].T
    # block 4: single (2,2), top half only
    wsb[0:64, 4 * OC:5 * OC] = Wf[:, :, 2, 2].T
    return wsb.astype(ml_dtypes.bfloat16)


def kernel(x: np.ndarray, Wt: np.ndarray) -> np.ndarray:
    assert x.shape == (8, IC, H, W) and Wt.shape == (OC, IC, K, K)
    if MODE not in _NC_CACHE:
        _NC_CACHE[MODE] = _build(MODE)
    nc = _NC_CACHE[MODE]

    wt_t = _pack_weights(Wt)
    in_maps = [
        {
            "xbf": np.ascontiguousarray(
                x[b].astype(ml_dtypes.bfloat16)
            ),
            "wt": wt_t,
        }
        for b in range(8)
    ]
    global _last_in_maps
    _last_in_maps = in_maps
    res = run_bass_kernel_spmd(nc, in_maps, core_ids=list(range(8)))
    out = np.stack([r["y"].reshape(OC, H, W) for r in res.results])
    return out.astype(np.float32)


_last_in_maps: list[dict[str, np.ndarray]] = []


# revision 23
# speedup vs baseline: 3.3569x; 1.0904x over previous
"""Trainium2 Bass kernel for nn_KeyedConv2d: 3x3 SAME conv, stride 1.

x: [8, 64, 64, 64] (NCHW), Wt: [64, 64, 3, 3] (OIHW) -> out [8, 64, 64, 64].

Sharding: data-parallel over batch, one image per NeuronCore (8 cores).

Per-core algorithm (v2):
- The 9 kernel offsets are covered by 6 matmuls per 512-pixel output chunk:
  3 "paired" matmuls with 128-deep contraction (offsets (ky,0)+(ky,1) fused:
  SBUF partitions 0-63 hold the padded image, partitions 64-127 hold the
  same image shifted one column, so one matmul contracts both offsets), plus
  3 single matmuls (ky,2) with 64-deep contraction.  Matmul cost on TRN2 is
  proportional to output free size only, so pairing halves PE time for the
  paired offsets.
- ~13 warmup matmuls on a zeroed tile keep the PE busy from ~0.3us so the
  pstate ramp reaches full clock (2.4 GHz) before the real matmuls' deps
  fire; instructions are costed at dep-ready time, so all real matmuls run
  at 213ns instead of 788ns.
- Inputs stream as 8 per-strip HBM->SBUF DMAs (contiguous >=512B elements,
  partition_broadcast duplicates the image into both partition halves);
  Activation/Pool engines convert+copy f32 staging into bf16 padded strip
  tiles (engines cannot cross partitions, hence the broadcast DMA).
- bf16 matmuls, f32 PSUM accumulation, direct PSUM->HBM stores (one PSUM
  bank per chunk, no eviction copy).
"""
import numpy as np
import ml_dtypes

import concourse.bass as bass
import concourse.mybir as mybir
import concourse.tile as tile
from concourse import bacc
from concourse.bass_utils import run_bass_kernel_spmd

F32 = mybir.dt.float32
BF16 = mybir.dt.bfloat16

IC = OC = 64
H = W = 64
K = 3
HWPIX = H * W        # 4096
CHUNK = 512          # output pixels per PSUM bank
NCH = HWPIX // CHUNK  # 8 chunks
RPC = CHUNK // W     # 8 image rows per chunk
PW = W + 1           # padded row width (left zero col; col 64 of the last
                     # window doubles as right pad via row contiguity)
SROWS = RPC + 2      # padded rows per strip incl halo (10)
SLEN = SROWS * PW + 2  # 652: singles' last 8x65 window ends at elem 651
RB = 656             # region B base: holds (A-top shifted 2, A-bot shifted
                     # 66) so one matmul pairs offsets (ky,2)+(ky+1,2)
RBLEN = 8 * PW       # 520
TLEN = RB + RBLEN    # strip tile total width

WARMN = 19           # warmup matmuls (tuned so warmup span ~ first chunk ready)

MODE = "bf16"


def _build(mode: str = MODE) -> bacc.Bacc:
    nc = bacc.Bacc("TRN2", target_bir_lowering=False, debug=False)

    # image pre-cast to bf16 on the host: halves the piece-DMA bytes (the
    # matmuls consume bf16 anyway)
    xbf = nc.dram_tensor("xbf", [IC, H, W], BF16, kind="ExternalInput").ap()
    # host-prepacked weights [128, 384] bf16:
    #   wt[ic,       ky*64+oc] = Wt[oc, ic, ky, 0]
    #   wt[64+ic,    ky*64+oc] = Wt[oc, ic, ky, 1]
    #   wt[ic, (3+ky)*64+oc]   = Wt[oc, ic, ky, 2]
    wt = nc.dram_tensor("wt", [128, 5 * OC], BF16, kind="ExternalInput").ap()
    y = nc.dram_tensor("y", [OC, HWPIX], BF16, kind="ExternalOutput").ap()

    with tile.TileContext(nc) as tc:
        with (
            tc.tile_pool(name="wsb", bufs=1) as wsb_pool,
            tc.tile_pool(name="warm", bufs=1) as warm_pool,
            tc.tile_pool(name="piece", bufs=1) as piece_pool,
            tc.tile_pool(name="xs", bufs=1) as xs_pool,
            tc.tile_pool(name="psum", bufs=1, space="PSUM") as psum_pool,
            tc.tile_pool(name="osb", bufs=6) as osb_pool,
        ):
            # --- warmup source (DVE memset, ready ~0.2us)
            warm = warm_pool.tile([64, 128], BF16)
            nc.vector.memset(warm[:, :], 0.0)

            # --- warmup matmuls: keep PE busy until first real chunk is
            # ready.  Shares PSUM bank (tag) with chunk 0; all deps are
            # PE-program-order so the sharing is free.
            wps = psum_pool.tile([64, CHUNK], F32, name="ps0")
            for _ in range(WARMN):
                nc.tensor.matmul(
                    wps[:, 0:128], warm[:, 0:64], warm[:, 0:128],
                    start=True, stop=True, skip_group_check=True,
                )

            # --- staging pieces: strip k needs image rows 8k-1 .. 8k+8.
            # Piece 0 is split in two and issued first so strip 0's pad
            # copies (and hence the first matmuls) start as early as
            # possible; the weights DMA slots between them.
            pieces = {}
            prows = {}
            for key, r0, r1 in (("0a", 0, 6), ("0b", 7, 8)):
                nr = r1 - r0 + 1
                pc = piece_pool.tile([128, nr * W], BF16, name=f"pc{key}")
                nc.sync.dma_start(
                    pc[:, :], xbf[:, r0:r1 + 1, :].partition_broadcast(2)
                )
                pieces[key] = pc
                prows[key] = (r0, nr)

            # --- weights via SWDGE (gpsimd): stays off the serial HWDGE
            # chain so the piece DMAs pipeline back-to-back
            wsb = wsb_pool.tile([128, 5 * OC], BF16)
            nc.gpsimd.dma_start(wsb[:, :], wt)

            for k in range(1, NCH):
                r0 = max(8 * k - 1, 0)
                r1 = min(8 * k + 8, H - 1)
                nr = r1 - r0 + 1
                pc = piece_pool.tile([128, nr * W], BF16, name=f"pc{k}")
                nc.sync.dma_start(
                    pc[:, :], xbf[:, r0:r1 + 1, :].partition_broadcast(2)
                )
                pieces[k] = pc
                prows[k] = (r0, nr)

            # --- strip tiles + zero slivers (DVE) + pad copies (Act top,
            #     Pool bottom, f32->bf16)
            xss = []
            for k in range(NCH):
                xs = xs_pool.tile([128, TLEN], BF16, name=f"xs{k}")
                xss.append(xs)
                # top half left-pad column (rows 0..9 plus the trailing
                # element 650 = "row 10 col 0" read by the ky=2 single)
                nc.vector.memset(
                    xs[0:64, 0:SROWS * PW].rearrange("p (a b) -> p a b", b=PW)[:, :, 0:1],
                    0.0,
                )
                nc.vector.memset(xs[0:64, SROWS * PW:SROWS * PW + 1], 0.0)
                # region B col 63 of each row is the right pad (zero); both
                # halves share the position
                nc.vector.memset(
                    xs[:, RB:RB + 8 * PW].rearrange(
                        "p (a b) -> p a b", b=PW)[:, :, W - 1:W],
                    0.0,
                )
                if k == 0:
                    # B-top row 0 = padded image row 0 (zeros)
                    nc.vector.memset(xs[0:64, RB:RB + W], 0.0)
                if k == 0:
                    nc.vector.memset(xs[:, 0:PW], 0.0)          # pad row 0
                if k == NCH - 1:
                    nc.vector.memset(xs[:, 9 * PW:9 * PW + PW], 0.0)  # pad row 65

            # Pad-copy jobs (strip, piece, rows, mode, engine).  Modes:
            # T/B = region-A top (cols 1..64) / bottom (cols 0..63, one col
            # left-shifted); BT/BB = region-B top/bottom, sourced straight
            # from the piece (img cols 1..63 at PW stride), so every copy
            # depends only on its piece DMA -- uniform readiness keeps the
            # static scheduler honest.  Region B reads at row base r then
            # contract offsets (0,2) [top] and (1,2) [bottom] in one matmul.
            # Strips 0-1 gate the PE pipeline start, so their copies are
            # split finely and lean on DVE's 2x 16-bit mode; steady-state
            # strips use Act (tops) and Pool (bottoms) in parallel with DVE
            # building region B.
            T, B, BT, BB = 0, 1, 2, 3
            copy_jobs = [
                (0, "0a", 1, 7, T, nc.scalar), (0, "0a", 1, 7, B, nc.vector),
                (0, "0a", 1, 7, BT, nc.vector), (0, "0a", 0, 6, BB, nc.vector),
                (0, "0b", 8, 9, T, nc.vector), (0, "0b", 8, 9, B, nc.vector),
                (0, "0b", 7, 7, BB, nc.vector),
                (1, 1, 0, 6, T, nc.scalar), (1, 1, 7, 9, T, nc.vector),
                (1, 1, 0, 9, B, nc.vector),
                (1, 1, 0, 7, BT, nc.vector), (1, 1, 0, 7, BB, nc.vector),
            ]
            for k in range(2, NCH):
                copy_jobs += [
                    (k, k, 0, 9 if k < NCH - 1 else 8, T, nc.scalar),
                    (k, k, 0, 9 if k < NCH - 1 else 8, B, nc.gpsimd),
                    (k, k, 0, 7, BT, nc.vector),
                    (k, k, 0, 7, BB, nc.vector),
                ]

            for k, key, rlo, rhi, mode, eng in copy_jobs:
                pc = pieces[key]
                xs = xss[k]
                r0, nr = prows[key]
                # A-strip row r / B-region row r hold padded row 8k+r, i.e.
                # img row 8k+r-1 (top) or 8k+r (B-bottom)
                off = 8 * k + rlo - 1 - r0 + (1 if mode == BB else 0)
                assert 0 <= off and off + (rhi - rlo) < nr
                src = pc[:, off * W:(off + rhi - rlo + 1) * W].rearrange(
                    "p (a b) -> p a b", b=W)
                rb = RB if mode in (BT, BB) else 0
                dst = xs[:, rb + rlo * PW:rb + (rhi + 1) * PW].rearrange(
                    "p (a b) -> p a b", b=PW)
                if mode == T:
                    if eng is nc.scalar:
                        eng.copy(dst[0:64, :, 1:1 + W], src[0:64, :, :])
                    else:
                        eng.tensor_copy(dst[0:64, :, 1:1 + W], src[0:64, :, :])
                elif mode == B:
                    eng.tensor_copy(dst[64:128, :, 0:W], src[64:128, :, :])
                elif mode == BT:
                    eng.tensor_copy(dst[0:64, :, 0:W - 1], src[0:64, :, 1:W])
                else:
                    eng.tensor_copy(dst[64:128, :, 0:W - 1], src[64:128, :, 1:W])

            # --- conv: per chunk, 3 paired + 3 single matmuls into one
            # PSUM bank.  Work items: (strip k, first strip row, npix, psum
            # tag).  Chunk 7 is split in two 256-px halves so the final
            # eviction+store tail after the last matmul is short.  The two
            # halves recycle the ps0/ps1 bank tags (long since evicted).
            work = [
                (k, 0, CHUNK, f"ps{k}", k % 2 == 1 or k == 6)
                for k in range(NCH - 1)
            ]
            work += [(7, 0, 6 * W, "ps0", False), (7, 6, 2 * W, "ps1", True)]
            evicted = []
            for k, rbase, npix, tag, flush in work:
                xs = xss[k]
                nrows = npix // W
                ps = psum_pool.tile([64, npix], F32, name=tag)
                for t, ky in enumerate(range(K)):
                    o = (rbase + ky) * PW
                    rhs = xs[:, o:o + nrows * PW].rearrange(
                        "p (a b) -> p a b", b=PW)[:, :, 0:W]
                    nc.tensor.matmul(
                        ps[:, :], wsb[:, ky * OC:(ky + 1) * OC], rhs,
                        start=(t == 0), stop=False, skip_group_check=True,
                    )
                # single (2,2) from region A's top half
                o = (rbase + 2) * PW + 2
                rhs = xs[0:64, o:o + nrows * PW].rearrange(
                    "p (a b) -> p a b", b=PW)[:, :, 0:W]
                nc.tensor.matmul(
                    ps[:, :], wsb[0:64, 4 * OC:5 * OC], rhs,
                    start=False, stop=False, skip_group_check=True,
                )
                # paired (0,2)+(1,2) from region B (last: B copies get the
                # longest overlap window behind the 4 region-A matmuls)
                o = RB + rbase * PW
                rhs = xs[:, o:o + nrows * PW].rearrange(
                    "p (a b) -> p a b", b=PW)[:, :, 0:W]
                nc.tensor.matmul(
                    ps[:, :], wsb[:, 3 * OC:4 * OC], rhs,
                    start=False, stop=True, skip_group_check=True,
                )
                # PSUM -> SBUF bf16 (DVE) eagerly per chunk; batched HBM
                # store at flush points (last flush = lone 128-px half-chunk
                # eviction, keeping the post-last-matmul tail short)
                if not evicted:
                    gbase = k * CHUNK + rbase * W
                    osb = osb_pool.tile([64, 2 * CHUNK], BF16, name="osb")
                # evictions alternate DVE/Act (both also feed pad/region-B
                # copies); the 6-row half-chunk lands on Act so only the
                # final 128-px eviction trails the last matmul
                odst = osb[:, sum(evicted):sum(evicted) + npix]
                if (k % 2 == 1 and k != 7) or (tag == "ps0" and k == 7):
                    nc.scalar.copy(odst, ps[:, :])
                else:
                    nc.vector.tensor_copy(odst, ps[:, :])
                evicted.append(npix)
                if flush:
                    tot = sum(evicted)
                    nc.sync.dma_start(
                        y[:, gbase:gbase + tot], osb[:, 0:tot]
                    )
                    evicted = []

    nc.compile()
    return nc


_NC_CACHE: dict[str, bacc.Bacc] = {}


def _pack_weights(Wt: np.ndarray) -> np.ndarray:
    Wf = Wt.astype(np.float32)
    wsb = np.zeros((128, 5 * OC), dtype=np.float32)
    # blocks 0-2: A-pairs (ky, kx=0) top / (ky, kx=1) bottom
    # [oc, ic, ky, kx] -> [ic, ky, oc]
    wsb[0:64, 0:K * OC] = Wf[:, :, :, 0].transpose(1, 2, 0).reshape(IC, K * OC)
    wsb[64:128, 0:K * OC] = Wf[:, :, :, 1].transpose(1, 2, 0).reshape(IC, K * OC)
    # block 3: B-pair (0,2) top / (1,2) bottom
    wsb[0:64, 3 * OC:4 * OC] = Wf[:, :, 0, 2].T
    wsb[64:128, 3 * OC:4 * OC] = Wf[:, :, 1, 2].T
    # block 4: single (2,2), top half only
    wsb[0:64, 4 * OC:5 * OC] = Wf[:, :, 2, 2].T
    return wsb.astype(ml_dtypes.bfloat16)


def kernel(x: np.ndarray, Wt: np.ndarray) -> np.ndarray:
    assert x.shape == (8, IC, H, W) and Wt.shape == (OC, IC, K, K)
    if MODE not in _NC_CACHE:
        _NC_CACHE[MODE] = _build(MODE)
    nc = _NC_CACHE[MODE]

    wt_t = _pack_weights(Wt)
    in_maps = [
        {
            "xbf": np.ascontiguousarray(
                x[b].astype(ml_dtypes.bfloat16)
            ),
            "wt": wt_t,
        }
        for b in range(8)
    ]
    global _last_in_maps
    _last_in_maps = in_maps
    res = run_bass_kernel_spmd(nc, in_maps, core_ids=list(range(8)))
    out = np.stack([r["y"].reshape(OC, H, W) for r in res.results])
    return out.astype(np.float32)


_last_in_maps: list[dict[str, np.ndarray]] = []
